# revision 1
# baseline (speedup 1.0000x reference)
"""Chamfer loss kernel for Trainium2 (8 NeuronCores, data-parallel over batch),
with host-side k-d-tree block pruning and per-core static schedules dispatched
by an 8-way Switch on partition id.

Math (per batch): P[i,j] = |x_i - y_j|^2; loss = sum_j min_i P + sum_i min_j P.
On device PN = -P/2 is computed by a K=21 matmul (bf16 hi/lo split, exact
products): loss = -2 * (sum_j max_i PN + sum_i max_j PN).

Pruning: points are reordered into 64 k-d leaves of 128 (free: chamfer is
permutation-invariant). For leaf pair (I,J) a rigorous bound excludes pairs
that provably contain no row- or column-argmin (point-to-box distance vs a
candidate NN upper bound). Only ~20% of leaf pairs survive. Each core's exact
schedule is emitted as its own Switch arm — fully static instruction stream,
shared tails after reconverge.
"""

import os
from contextlib import ExitStack

import ml_dtypes
import numpy as np

import concourse.bacc as bacc
import concourse.bass as bass
import concourse.mybir as mybir
import concourse.tile as tile
from concourse.bass_utils import run_bass_kernel_spmd

B, D, N = 8, 3, 8192
BLK = 128
NB = N // BLK          # 64 leaves
N_CORES = 8
KROWS = 21             # hi/lo split contraction rows
QW = 2048              # PSUM quad width (4 banks)
BANK = 512             # one PSUM bank of fp32
ROWSLOTS = 32          # rowpart slots per leaf (strip stride)
NEG = -60000.0         # -inf surrogate valid in fp16

F32 = mybir.dt.float32
F16 = mybir.dt.float16
BF16 = mybir.dt.bfloat16
AX = mybir.AxisListType
ALU = mybir.AluOpType

ROW_MODE = os.environ.get("CHAMFER_ROW", "ts")    # ts | red  (ttr faults HW)
GAP = int(os.environ.get("CHAMFER_GAP", "1"))

_last_results = None


# ---------------------------------------------------------------------------
# host-side schedule construction
# ---------------------------------------------------------------------------

def _kd_order(p):
    """Permutation putting points into 64 kd leaves of 128, DFS order."""
    out = []

    def rec(ids):
        if len(ids) <= BLK:
            out.append(ids)
            return
        q = p[ids]
        d = np.argmax(q.max(0) - q.min(0))
        order = np.argsort(q[:, d], kind="stable")
        half = len(ids) // (2 * BLK) * BLK
        if half == 0:
            half = len(ids) // 2
        rec(ids[order[:half]])
        rec(ids[order[half:]])

    rec(np.arange(len(p)))
    return np.concatenate(out)


def _point_box_d2(pts, blo, bhi):
    g = np.maximum(0, np.maximum(blo[None, :, :] - pts[:, None, :],
                                 pts[:, None, :] - bhi[None, :, :]))
    return (g ** 2).sum(-1)


def _batch_runs(x, y, cand_boxes=4, gap=GAP):
    """kd-sort both point sets; return (xs, ys, runs) with runs a list of
    (I, j0, len_blocks) covering every block that can hold a row/col argmin."""
    xs = x[_kd_order(x)]
    ys = y[_kd_order(y)]
    xlo = xs.reshape(NB, BLK, 3).min(1); xhi = xs.reshape(NB, BLK, 3).max(1)
    ylo = ys.reshape(NB, BLK, 3).min(1); yhi = ys.reshape(NB, BLK, 3).max(1)

    dxb = _point_box_d2(xs, ylo, yhi)          # [N, NB]
    u = np.full(N, np.inf)
    nearest = np.argsort(dxb, axis=1)[:, :cand_boxes]
    for c in range(cand_boxes):
        J = nearest[:, c]
        for Jv in np.unique(J):
            rows = np.nonzero(J == Jv)[0]
            d = ((xs[rows, None, :] - ys[Jv*BLK:(Jv+1)*BLK][None]) ** 2).sum(-1)
            u[rows] = np.minimum(u[rows], d.min(1))
    dyb = _point_box_d2(ys, xlo, xhi)
    v = np.full(N, np.inf)
    nearestx = np.argsort(dyb, axis=1)[:, :cand_boxes]
    for c in range(cand_boxes):
        I = nearestx[:, c]
        for Iv in np.unique(I):
            rows = np.nonzero(I == Iv)[0]
            d = ((ys[rows, None, :] - xs[Iv*BLK:(Iv+1)*BLK][None]) ** 2).sum(-1)
            v[rows] = np.minimum(v[rows], d.min(1))

    row_need = (dxb <= u[:, None]).reshape(NB, BLK, NB).any(1)
    col_need = (dyb <= v[:, None]).reshape(NB, BLK, NB).any(1)
    need = row_need | col_need.T

    runs = []
    for I in range(NB):
        js = np.nonzero(need[I])[0]
        start = prev = js[0]
        for j in js[1:]:
            if j - prev <= gap + 1:
                prev = j
            else:
                runs.append((I, int(start), int(prev - start + 1)))
                start = prev = j
        runs.append((I, int(start), int(prev - start + 1)))
    return xs, ys, runs


def _hi_lo(a):
    hi = a.astype(ml_dtypes.bfloat16)
    lo = (a - hi.astype(np.float32)).astype(ml_dtypes.bfloat16)
    return hi, lo


def _pack21_x(p):
    """[n,3] points -> [21,n] bf16 lhsT rows:
    0-2 hx, 3-5 hx, 6-8 lx, 9-11 hsq, 12-14 lsq, 15-20 -1/2."""
    a = p.T.astype(np.float32)
    hx, lx = _hi_lo(a)
    hs, ls = _hi_lo(a * a)
    out = np.empty((KROWS, p.shape[0]), dtype=ml_dtypes.bfloat16)
    out[0:3] = hx; out[3:6] = hx; out[6:9] = lx
    out[9:12] = hs; out[12:15] = ls
    out[15:21] = np.float32(-0.5)
    return out


def _pack21_y(p):
    """rhs rows: 0-2 hy, 3-5 ly, 6-8 hy, 9-14 -1/2, 15-17 hsq, 18-20 lsq."""
    a = p.T.astype(np.float32)
    hy, ly = _hi_lo(a)
    hs, ls = _hi_lo(a * a)
    out = np.empty((KROWS, p.shape[0]), dtype=ml_dtypes.bfloat16)
    out[0:3] = hy; out[3:6] = ly; out[6:9] = hy
    out[9:15] = np.float32(-0.5)
    out[15:18] = hs; out[18:21] = ls
    return out


def build_schedule(preds, gts):
    """Per-core run lists + packed input tensors."""
    plans, tensors = [], []
    for b in range(B):
        xs, ys, runs = _batch_runs(preds[b].T, gts[b].T)
        plans.append(runs)
        tensors.append((np.ascontiguousarray(_pack21_x(xs)),
                        np.ascontiguousarray(_pack21_y(ys))))
    return plans, tensors


# ---------------------------------------------------------------------------
# device kernel
# ---------------------------------------------------------------------------

def _plan_quads(runs):
    """Chop a core's run list into PSUM-quad fragments and bank pieces.
    Returns (nq, frags, pieces):
      frags: per quad, list of (I, jcol, off_in_quad, wid) for DVE ops
      pieces: per quad, list of (I, jcol, off_in_quad, wid) for matmuls
    jcol = absolute starting column in YT/C for the fragment/piece."""
    frags, pieces = [], []
    pos = 0
    for (I, j0, fd) in runs:
        w = fd * BLK
        c = 0
        while c < w:
            take = min(w - c, QW - (pos % QW))
            frags.append((I, j0 * BLK + c, pos // QW, pos % QW, take))
            c += take
            pos += take
    nq = (pos + QW - 1) // QW
    for (I, jcol, q, off, wid) in frags:
        c = 0
        while c < wid:
            take = min(wid - c, BANK - ((off + c) % BANK))
            pieces.append((I, jcol + c, q, off + c, take))
            c += take
    byq_f = [[] for _ in range(nq)]
    for (I, jcol, q, off, wid) in frags:
        byq_f[q].append((I, jcol, off, wid))
    byq_p = [[] for _ in range(nq)]
    for (I, jcol, q, off, wid) in pieces:
        byq_p[q].append((I, jcol, off, wid))
    qw_last = pos - (nq - 1) * QW
    return nq, byq_f, byq_p, qw_last


def build_kernel(plans):
    nc = bacc.Bacc("TRN2", target_bir_lowering=False, debug=False)

    xt_d = nc.dram_tensor("xt", [KROWS, N], BF16, kind="ExternalInput").ap()
    yt_d = nc.dram_tensor("yt", [KROWS, N], BF16, kind="ExternalInput").ap()
    ident_d = nc.dram_tensor("ident", [128, 128], F16, kind="ExternalInput").ap()
    out_d = nc.dram_tensor("out", [1, 1], F32, kind="ExternalOutput").ap()

    core_quads = [_plan_quads(runs) for runs in plans]

    with tile.TileContext(nc) as tc, ExitStack() as ctx:
        persist = ctx.enter_context(tc.tile_pool(name="persist", bufs=1))
        spool = ctx.enter_context(tc.tile_pool(name="spool", bufs=3))
        dpool = ctx.enter_context(tc.tile_pool(name="dpool", bufs=2))
        psum_ctx = tc.tile_pool(name="psum", bufs=2, space=bass.MemorySpace.PSUM)
        psum = psum_ctx.__enter__()

        XT = persist.tile([KROWS, N], BF16)
        YT = persist.tile([KROWS, N], BF16)
        ident = persist.tile([128, 128], F16)
        C = persist.tile([128, N], F16)
        rowstrip = persist.tile([128, NB * ROWSLOTS], F32)

        nc.sync.dma_start(XT[:], xt_d[:])
        nc.sync.dma_start(YT[:], yt_d[:])
        nc.sync.dma_start(ident[:], ident_d[:])
        nc.gpsimd.memset(C[:], NEG)
        nc.vector.memset(rowstrip[:], NEG)

        engines = (mybir.EngineType.PE, mybir.EngineType.Activation,
                   mybir.EngineType.DVE)
        pid = nc.partition_id(engines=engines)

        for arm in tc.Switch(pid, N_CORES):
            nq, byq_f, byq_p, qw_last = core_quads[arm]
            slot_cnt = {}
            for q in range(nq):
                qw = QW if q < nq - 1 else qw_last
                p = psum.tile([128, QW], F32, tag="p")
                for (I, jcol, off, wid) in byq_p[q]:
                    nc.tensor.matmul(
                        p[:, off:off+wid],
                        XT[:, I*BLK:(I+1)*BLK],
                        YT[:, jcol:jcol+wid],
                        start=True, stop=True)
                s = spool.tile([128, QW], F16, tag="s")
                nc.scalar.copy(s[:, 0:qw], p[:, 0:qw])
                for (I, jcol, off, wid) in byq_f[q]:
                    cdst = C[:, jcol:jcol+wid]
                    nc.vector.tensor_tensor(out=cdst, in0=cdst,
                                            in1=s[:, off:off+wid], op=ALU.max)
                    k = slot_cnt.get(I, 0)
                    assert k < ROWSLOTS, f"leaf {I} overflows rowslots"
                    slot_cnt[I] = k + 1
                    acc = rowstrip[:, I*ROWSLOTS + k: I*ROWSLOTS + k + 1]
                    if ROW_MODE == "red":
                        nc.vector.tensor_reduce(
                            out=acc, in_=s[:, off:off+wid],
                            axis=AX.X, op=ALU.max)
                    elif ROW_MODE == "ts":
                        dead = dpool.tile([128, QW], F16, tag="dead")
                        nc.vector.tensor_scalar(
                            out=dead[:, 0:wid], in0=s[:, off:off+wid],
                            scalar1=0.0, scalar2=None,
                            op0=ALU.add, op1=ALU.max, accum_out=acc)
                    else:
                        dead = dpool.tile([128, QW], F16, tag="dead")
                        nc.vector.tensor_tensor_reduce(
                            out=dead[:, 0:wid], in0=s[:, off:off+wid],
                            in1=s[:, off:off+wid], scale=1.0, scalar=NEG,
                            op0=ALU.max, op1=ALU.max, accum_out=acc)

        # ---- tails (shared) ----
        psum_ctx.__exit__(None, None, None)
        tailp = ctx.enter_context(
            tc.tile_pool(name="tailp", bufs=2, space=bass.MemorySpace.PSUM))

        rowred = persist.tile([128, NB], F32)
        nc.vector.tensor_reduce(
            out=rowred[:],
            in_=rowstrip[:].rearrange("p (i s) -> p i s", s=ROWSLOTS),
            axis=AX.X, op=ALU.max)
        acc2 = persist.tile([128, 1], F32)
        nc.vector.reduce_sum(out=acc2[:], in_=rowred[:], axis=AX.X)

        n_cols = N // 128
        colmax_cols = persist.tile([128, n_cols], F32)
        for g in range(n_cols // 4):
            pt = tailp.tile([128, 512], F16, tag="pt")
            for c4 in range(4):
                ch = g * 4 + c4
                nc.tensor.transpose(
                    pt[:, c4*128:(c4+1)*128],
                    C[:, ch*128:(ch+1)*128], ident[:])
            nc.vector.tensor_reduce(
                out=colmax_cols[:, g*4:(g+1)*4],
                in_=pt[:].rearrange("p (c f) -> p c f", c=4),
                axis=AX.X, op=ALU.max)
        acc1 = persist.tile([128, 1], F32)
        nc.vector.reduce_sum(out=acc1[:], in_=colmax_cols[:], axis=AX.X)

        total = persist.tile([128, 1], F32)
        nc.vector.tensor_tensor(out=total[:], in0=acc1[:], in1=acc2[:], op=ALU.add)
        ones = persist.tile([128, 1], F32)
        nc.vector.memset(ones[:], 1.0)
        ps = tailp.tile([1, 1], F32, tag="ps")
        nc.tensor.matmul(ps[:], ones[:], total[:], start=True, stop=True)
        out_sb = persist.tile([1, 1], F32)
        nc.scalar.mul(out_sb[:], ps[:], -2.0)
        nc.sync.dma_start(out_d[:], out_sb[:])

    nc.compile()
    return nc


def kernel(preds: np.ndarray, gts: np.ndarray) -> np.ndarray:
    global _last_results
    assert preds.shape == (B, D, N) and gts.shape == (B, D, N)
    preds = np.asarray(preds, dtype=np.float32)
    gts = np.asarray(gts, dtype=np.float32)

    plans, tensors = build_schedule(preds, gts)
    nc = build_kernel(plans)
    eye = np.eye(128, dtype=np.float16)
    in_maps = [
        {"xt": xt, "yt": yt, "ident": eye}
        for (xt, yt) in tensors
    ]
    res = run_bass_kernel_spmd(
        nc,
        in_maps,
        core_ids=list(range(N_CORES)),
        trace=bool(os.environ.get("BASS_TRACE")),
    )
    _last_results = res
    total = sum(float(res.results[i]["out"].reshape(-1)[0]) for i in range(N_CORES))
    return np.array(total, dtype=np.float32)



# revision 7
# speedup vs baseline: 2.1243x; 2.1243x over previous
"""Chamfer loss kernel for Trainium2 (8 NeuronCores, data-parallel over batch),
with host-side k-d-tree block pruning and per-core static schedules dispatched
by an 8-way Switch on partition id.

Math (per batch): P[i,j] = |x_i - y_j|^2; loss = sum_j min_i P + sum_i min_j P.
On device PN = -P/2 is computed by a K=13 matmul (bf16 hi/lo split plus
presummed hi/lo squared norms; exact to ~2^-16): loss = -2 * sum of row maxes.

Dual-rowmax structure: BOTH loss terms are computed as row-max reductions.
  Side A (loss_2, min over gts per pred): stationary = 128-point x-leaf,
    moving = that leaf's candidate y columns (32-point block granularity).
  Side B (loss_1, min over preds per gt): stationary = 128-point y-leaf,
    moving = candidate x columns. (PN is symmetric in construction.)
No column-max matrix, no transposes: the only reduction is the DVE
tensor_scalar accumulate-max over each leaf's contiguous packed span.

Pruning: points are kd-ordered into 64 leaves of 128, each nested into 4
sub-blocks of 32 (free: chamfer is permutation-invariant). A moving 32-block
is included for a stationary leaf iff some point of the leaf has
point-to-box distance <= that point's candidate-NN upper bound (rigorous).
"""

import os
from contextlib import ExitStack

import ml_dtypes
import numpy as np

import concourse.bacc as bacc
import concourse.bass as bass
import concourse.mybir as mybir
import concourse.tile as tile
from concourse.bass_utils import run_bass_kernel_spmd

B, D, N = 8, 3, 8192
BLK = 128              # stationary leaf size (PE partition dim)
SUB = 32               # moving-block granularity
NB = N // BLK          # 64 stationary leaves per side
NSB = N // SUB         # 256 moving blocks per side
N_CORES = 8
KROWS = 13             # hi/lo split contraction rows (presummed sq norms)
QW = 2048              # PSUM quad width (4 banks)
BANK = 512             # one PSUM bank of fp32
ROWSLOTS = 4           # rowmax slots per stationary leaf
NEG = -60000.0         # -inf surrogate valid in fp16

F32 = mybir.dt.float32
F16 = mybir.dt.float16
BF16 = mybir.dt.bfloat16
AX = mybir.AxisListType
ALU = mybir.AluOpType

GAP = int(os.environ.get("CHAMFER_GAP", "0"))
CAND = int(os.environ.get("CHAMFER_CAND", "4"))

_last_results = None


# ---------------------------------------------------------------------------
# host-side schedule construction
# ---------------------------------------------------------------------------

def _kd_order(p, blk):
    """Permutation putting points into leaves of `blk`, DFS order."""
    out = []

    def rec(ids, b):
        if len(ids) <= b:
            out.append(ids)
            return
        q = p[ids]
        d = np.argmax(q.max(0) - q.min(0))
        order = np.argsort(q[:, d], kind="stable")
        half = len(ids) // (2 * b) * b
        if half == 0:
            half = len(ids) // 2
        rec(ids[order[:half]], b)
        rec(ids[order[half:]], b)

    rec(np.arange(len(p)), blk)
    return np.concatenate(out)


def _kd_order_nested(p):
    """kd order to 128-leaves, each further kd-split into 32-blocks."""
    coarse = _kd_order(p, BLK)
    out = []
    for i in range(len(p) // BLK):
        ids = coarse[i * BLK:(i + 1) * BLK]
        sub = _kd_order(p[ids], SUB)
        out.append(ids[sub])
    return np.concatenate(out)


def _point_box_d2(pts, blo, bhi):
    g = np.maximum(0, np.maximum(blo[None, :, :] - pts[:, None, :],
                                 pts[:, None, :] - bhi[None, :, :]))
    return (g ** 2).sum(-1)


def _side_runs(pr, pc, gap=GAP, cand=CAND):
    """Stationary points pr (kd-ordered, 128-leaves), moving points pc
    (kd-ordered, 32-blocks). Returns runs [(I, j0, len_blocks)] covering,
    for every 128-leaf I, every 32-block that can hold a row argmin."""
    sub_lo = pc.reshape(NSB, SUB, 3).min(1)
    sub_hi = pc.reshape(NSB, SUB, 3).max(1)
    d_sub = _point_box_d2(pr, sub_lo, sub_hi)          # [N, NSB]
    u = np.full(N, np.inf)
    nearest = np.argsort(d_sub, axis=1)[:, :cand]
    for c in range(cand):
        J = nearest[:, c]
        for Jv in np.unique(J):
            rows = np.nonzero(J == Jv)[0]
            d = ((pr[rows, None, :] - pc[Jv*SUB:(Jv+1)*SUB][None]) ** 2).sum(-1)
            u[rows] = np.minimum(u[rows], d.min(1))
    need = (d_sub <= u[:, None]).reshape(NB, BLK, NSB).any(1)   # [NB, NSB]

    runs = []
    for I in range(NB):
        js = np.nonzero(need[I])[0]
        start = prev = js[0]
        for j in js[1:]:
            if j - prev <= gap + 1:
                prev = j
            else:
                runs.append((I, int(start), int(prev - start + 1)))
                start = prev = j
        runs.append((I, int(start), int(prev - start + 1)))
    return runs


def _hi_lo(a):
    hi = a.astype(ml_dtypes.bfloat16)
    lo = (a - hi.astype(np.float32)).astype(ml_dtypes.bfloat16)
    return hi, lo


def _pack13_lhs(p):
    """[n,3] points -> [13,n] bf16 stationary rows:
    0-2 h, 3-5 h, 6-8 l, 9 h|p|^2, 10 l|p|^2, 11-12 -1/2."""
    a = p.T.astype(np.float32)
    h, l = _hi_lo(a)
    hs, ls = _hi_lo((a.astype(np.float64) ** 2).sum(0).astype(np.float32))
    out = np.empty((KROWS, p.shape[0]), dtype=ml_dtypes.bfloat16)
    out[0:3] = h; out[3:6] = h; out[6:9] = l
    out[9] = hs; out[10] = ls
    out[11:13] = np.float32(-0.5)
    return out


def _pack13_rhs(p):
    """moving rows: 0-2 h, 3-5 l, 6-8 h, 9-10 -1/2, 11 h|p|^2, 12 l|p|^2."""
    a = p.T.astype(np.float32)
    h, l = _hi_lo(a)
    hs, ls = _hi_lo((a.astype(np.float64) ** 2).sum(0).astype(np.float32))
    out = np.empty((KROWS, p.shape[0]), dtype=ml_dtypes.bfloat16)
    out[0:3] = h; out[3:6] = l; out[6:9] = h
    out[9:11] = np.float32(-0.5)
    out[11] = hs; out[12] = ls
    return out


def build_schedule(preds, gts):
    """Per-core run lists (side A then side B) + packed input tensors."""
    plans, tensors = [], []
    for b in range(B):
        x = preds[b].T
        y = gts[b].T
        xs = x[_kd_order_nested(x)]
        ys = y[_kd_order_nested(y)]
        runs_a = _side_runs(xs, ys)          # loss_2: x rows, y cols
        runs_b = _side_runs(ys, xs)          # loss_1: y rows, x cols
        plans.append((runs_a, runs_b))
        tensors.append((
            np.ascontiguousarray(_pack13_lhs(xs)),
            np.ascontiguousarray(_pack13_rhs(ys)),
            np.ascontiguousarray(_pack13_lhs(ys)),
            np.ascontiguousarray(_pack13_rhs(xs)),
        ))
    return plans, tensors


# ---------------------------------------------------------------------------
# device kernel
# ---------------------------------------------------------------------------

def _plan_quads(plan):
    """Pack both sides' runs into PSUM quads.
    Returns (nq, byq_mm, byq_rx, qw_last):
      byq_mm: per quad, list of (side, I, jcol, off, wid) matmul pieces
              (bank-clipped; jcol/wid in columns)
      byq_rx: per quad, list of (side, I, off, wid) merged rowmax spans
    """
    runs_a, runs_b = plan
    stream = [(0, I, j0 * SUB, fd * SUB) for (I, j0, fd) in runs_a] + \
             [(1, I, j0 * SUB, fd * SUB) for (I, j0, fd) in runs_b]

    frags = []
    pos = 0
    for (side, I, jcol, w) in stream:
        c = 0
        while c < w:
            take = min(w - c, QW - (pos % QW))
            frags.append((side, I, jcol + c, pos // QW, pos % QW, take))
            c += take
            pos += take
    nq = (pos + QW - 1) // QW
    qw_last = pos - (nq - 1) * QW

    byq_mm = [[] for _ in range(nq)]
    for (side, I, jcol, q, off, wid) in frags:
        c = 0
        while c < wid:
            take = min(wid - c, BANK - ((off + c) % BANK))
            byq_mm[q].append((side, I, jcol + c, off + c, take))
            c += take

    byq_rx = [[] for _ in range(nq)]
    for (side, I, jcol, q, off, wid) in frags:
        rx = byq_rx[q]
        if rx and rx[-1][0] == side and rx[-1][1] == I and \
                rx[-1][2] + rx[-1][3] == off:
            rx[-1] = (side, I, rx[-1][2], rx[-1][3] + wid)
        else:
            rx.append((side, I, off, wid))
    return nq, byq_mm, byq_rx, qw_last


def build_kernel(plans):
    nc = bacc.Bacc("TRN2", target_bir_lowering=False, debug=False)

    xl_d = nc.dram_tensor("xl", [KROWS, N], BF16, kind="ExternalInput").ap()
    yr_d = nc.dram_tensor("yr", [KROWS, N], BF16, kind="ExternalInput").ap()
    yl_d = nc.dram_tensor("yl", [KROWS, N], BF16, kind="ExternalInput").ap()
    xr_d = nc.dram_tensor("xr", [KROWS, N], BF16, kind="ExternalInput").ap()
    out_d = nc.dram_tensor("out", [1, 1], F32, kind="ExternalOutput").ap()

    core_quads = [_plan_quads(plan) for plan in plans]

    with tile.TileContext(nc) as tc, ExitStack() as ctx:
        persist = ctx.enter_context(tc.tile_pool(name="persist", bufs=1))
        spool = ctx.enter_context(tc.tile_pool(name="spool", bufs=3))
        dpool = ctx.enter_context(tc.tile_pool(name="dpool", bufs=2))
        psum_ctx = tc.tile_pool(name="psum", bufs=2, space=bass.MemorySpace.PSUM)
        psum = psum_ctx.__enter__()

        XL = persist.tile([KROWS, N], BF16)
        YR = persist.tile([KROWS, N], BF16)
        YL = persist.tile([KROWS, N], BF16)
        XR = persist.tile([KROWS, N], BF16)
        LHS = (XL, YL)
        RHS = (YR, XR)
        # rowmax strip: leaves 0-63 side A, 64-127 side B
        rowstrip = persist.tile([128, 2 * NB * ROWSLOTS], F32)

        nc.sync.dma_start(XL[:], xl_d[:])
        nc.sync.dma_start(YR[:], yr_d[:])
        nc.sync.dma_start(YL[:], yl_d[:])
        nc.sync.dma_start(XR[:], xr_d[:])
        nc.vector.memset(rowstrip[:], NEG)

        engines = (mybir.EngineType.PE, mybir.EngineType.Activation,
                   mybir.EngineType.DVE)
        pid = nc.partition_id(engines=engines)

        for arm in tc.Switch(pid, N_CORES):
            nq, byq_mm, byq_rx, qw_last = core_quads[arm]
            slot_cnt = {}
            for q in range(nq):
                qw = QW if q < nq - 1 else qw_last
                p = psum.tile([128, QW], F32, tag="p")
                for (side, I, jcol, off, wid) in byq_mm[q]:
                    nc.tensor.matmul(
                        p[:, off:off+wid],
                        LHS[side][:, I*BLK:(I+1)*BLK],
                        RHS[side][:, jcol:jcol+wid],
                        start=True, stop=True)
                s = spool.tile([128, QW], F16, tag="s")
                nc.scalar.copy(s[:, 0:qw], p[:, 0:qw])
                for (side, I, off, wid) in byq_rx[q]:
                    leaf = side * NB + I
                    k = slot_cnt.get(leaf, 0)
                    assert k < ROWSLOTS, f"leaf {leaf} overflows rowslots"
                    slot_cnt[leaf] = k + 1
                    acc = rowstrip[:, leaf*ROWSLOTS + k: leaf*ROWSLOTS + k + 1]
                    dead = dpool.tile([128, QW], F16, tag="dead")
                    nc.vector.tensor_scalar(
                        out=dead[:, 0:wid], in0=s[:, off:off+wid],
                        scalar1=0.0, scalar2=None,
                        op0=ALU.add, op1=ALU.max, accum_out=acc)

        # ---- tail (shared) ----
        psum_ctx.__exit__(None, None, None)
        tailp = ctx.enter_context(
            tc.tile_pool(name="tailp", bufs=1, space=bass.MemorySpace.PSUM))

        rowred = persist.tile([128, 2 * NB], F32)
        nc.vector.tensor_reduce(
            out=rowred[:],
            in_=rowstrip[:].rearrange("p (i s) -> p i s", s=ROWSLOTS),
            axis=AX.X, op=ALU.max)
        acc = persist.tile([128, 1], F32)
        nc.vector.reduce_sum(out=acc[:], in_=rowred[:], axis=AX.X)

        ones = persist.tile([128, 1], F32)
        nc.vector.memset(ones[:], 1.0)
        ps = tailp.tile([1, 1], F32, tag="ps")
        nc.tensor.matmul(ps[:], ones[:], acc[:], start=True, stop=True)
        out_sb = persist.tile([1, 1], F32)
        nc.scalar.mul(out_sb[:], ps[:], -2.0)
        nc.sync.dma_start(out_d[:], out_sb[:])

    nc.compile()
    return nc


def kernel(preds: np.ndarray, gts: np.ndarray) -> np.ndarray:
    global _last_results
    assert preds.shape == (B, D, N) and gts.shape == (B, D, N)
    preds = np.asarray(preds, dtype=np.float32)
    gts = np.asarray(gts, dtype=np.float32)

    plans, tensors = build_schedule(preds, gts)
    nc = build_kernel(plans)
    in_maps = [
        {"xl": xl, "yr": yr, "yl": yl, "xr": xr}
        for (xl, yr, yl, xr) in tensors
    ]
    res = run_bass_kernel_spmd(
        nc,
        in_maps,
        core_ids=list(range(N_CORES)),
        trace=bool(os.environ.get("BASS_TRACE")),
    )
    _last_results = res
    total = sum(float(res.results[i]["out"].reshape(-1)[0]) for i in range(N_CORES))
    return np.array(total, dtype=np.float32)
